# revision 17
# baseline (speedup 1.0000x reference)
"""GatedGCN (2-layer) Trainium2 Bass kernel, 8 NeuronCores, full-I/O contract.

Strategy (1D destination-node graph partition), v5:
  - Associativity: A@(x@W) == (A@x)@W, so each layer computes z = A@x first
    (256-wide gather+aggregation) and applies [W|G] afterwards.  Layer 1
    gathers straight from the replicated input x, so only the single h
    AllGather (between the layers) remains.
  - Pad N=50000 rows to 50176 = 8 cores * 6272 rows; core c owns rows
    [c*6272, (c+1)*6272) and every edge whose DEST row lands there.
  - Host sorts edges by destination window (128 dest rows per window, 49
    windows per core), lo (col < 32768) before hi, and pads each group to
    a uniform (across cores) number of 128-edge tiles with val=0 edges.
  - Host precomputes the per-tile selection matrices
    sel[e, r] = vals[e] * (dest_local[e] == r)  (bf16, shipped as input,
    streamed from DRAM each layer) - no on-device sel builds.
  - Aggregation per window: dma_gather pulls msg = table[cols] into SBUF
    (lo/hi split for int16 indices; 4 SWDGE queues round-robin), then per
    128-edge tile one PE matmul ps += sel_t.T @ msg_t accumulates
    z[dest, 256] in PSUM.
  - z is copied to SBUF (scalar engine), PE-transposed to zT, and the
    dense stage ps2 = zT.T @ [W|G] + sigmoid-gate product follows, all
    fused per window.
  - The first PREFIX windows of layer 2 use prepare_only gathers: their
    descriptor generation runs on the gpsimd engine DURING the AllGather
    (the gather-table data dep defers to trigger_dma), so the collective
    bubble is spent generating descriptors instead of idling.
"""

import os

import numpy as np
import ml_dtypes

import concourse.bass as bass
import concourse.bacc as bacc
import concourse.mybir as mybir
import concourse.tile as tile
from concourse.bass_utils import run_bass_kernel_spmd

P = 128
NCORES = 8
N = 50000
D0, D1, D2 = 256, 256, 128
NWIN = 49
NC_ROWS = NWIN * P            # 6272 rows per core (padded)
NP = NC_ROWS * NCORES         # 50176 padded rows total
SG1 = 2 * D1                  # 512: [support | gate] layer 1
SG2 = 2 * D2                  # 256: [support | gate] layer 2
KC1 = D0 // P                 # k-chunks of z for the layer-1 dense matmul
KC2 = D1 // P
SPLIT = 32768                 # int16 gather-index range boundary
NQ = 4                        # SWDGE queues, round-robin over gather calls
PREFIX = 0                    # layer-2 windows whose desc-gen hides under AG

BF16 = ml_dtypes.bfloat16
F32 = mybir.dt.float32
BF = mybir.dt.bfloat16
I16 = mybir.dt.int16

LAST_RESULTS = None  # test harness reads exec_time_ns from here


# --------------------------------------------------------------------------
# Host-side edge plan
# --------------------------------------------------------------------------

def build_edge_plan(rows, cols, vals):
    """Sort edges by (core, window) of their dest row. Within a window, lo
    edges (col < SPLIT) come first, then hi edges; each group is padded to a
    uniform (across cores) number of 128-edge tiles.

    Returns (T, TL, colsA, valsA, dstlA):
      T[w]   total tiles in window w;  TL[w]  lo tiles (hi = T - TL)
      colsA  [NCORES, P, Ttot] int32   source row (global padded space)
      valsA  [NCORES, P, Ttot] float32 edge weight (0 for padding)
      dstlA  [NCORES, P, Ttot] float32 dest row local to window (0..127)
    Edge (window w, tile t, partition p) lives at [:, p, off[w]+t].
    """
    hi = (cols >= SPLIT).astype(np.int64)
    gw = (rows // P).astype(np.int64)               # global window
    order = np.argsort(gw * 2 + hi, kind="stable")  # window-major, lo first
    srows = rows[order]
    scols = cols[order]
    svals = vals[order]
    cnt = np.bincount(gw * 2 + hi, minlength=NCORES * NWIN * 2)
    cnt_cwh = cnt.reshape(NCORES, NWIN, 2)
    TL = np.ceil(cnt_cwh[:, :, 0] / P).astype(np.int64).max(axis=0)
    TH = np.ceil(cnt_cwh[:, :, 1] / P).astype(np.int64).max(axis=0)
    TL = np.maximum(TL, (TH == 0).astype(np.int64))  # >=1 tile per window
    T = TL + TH
    off = np.zeros(NWIN + 1, np.int64)
    off[1:] = np.cumsum(T)
    Ttot = int(off[-1])
    colsA = np.zeros((NCORES, P, Ttot), np.int32)
    # hi-group padding must point into the hi table: col = SPLIT
    for w in range(NWIN):
        if T[w] - TL[w] > 0:
            colsA[:, :, off[w] + TL[w]:off[w + 1]] = SPLIT
    valsA = np.zeros((NCORES, P, Ttot), np.float32)
    dstlA = np.zeros((NCORES, P, Ttot), np.float32)
    starts = np.zeros(NCORES * NWIN * 2 + 1, np.int64)
    starts[1:] = np.cumsum(cnt)
    for c in range(NCORES):
        for w in range(NWIN):
            for h in range(2):
                g = (c * NWIN + w) * 2 + h
                s, e = int(starts[g]), int(starts[g + 1])
                n = e - s
                if n == 0:
                    continue
                base = off[w] + (0 if h == 0 else TL[w])
                j = np.arange(n)
                t_idx = j // P
                p_idx = j % P
                colsA[c, p_idx, base + t_idx] = scols[s:e]
                valsA[c, p_idx, base + t_idx] = svals[s:e]
                dstlA[c, p_idx, base + t_idx] = srows[s:e] % P
    return T, TL, colsA, valsA, dstlA


def build_idx16(colsA, T, TL):
    """Wrapped int16 gather indices: for each window w and group q (lo/hi),
    gather j reads idx16[j % 16, base*8 + j // 16]; replicated to all 128
    partitions. Group hi indices are rebased by -SPLIT."""
    Ttot = int(np.sum(T))
    idx16 = np.zeros((NCORES, 16, Ttot * 8), np.int16)
    off = np.zeros(len(T) + 1, np.int64)
    off[1:] = np.cumsum(T)
    for c in range(NCORES):
        flat = np.ascontiguousarray(colsA[c].T)  # [Ttot, P]: edge j of window
        for w in range(len(T)):
            for (t0, t1, rebase) in (
                (off[w], off[w] + TL[w], 0),
                (off[w] + TL[w], off[w + 1], SPLIT),
            ):
                n = int((t1 - t0) * P)
                if n == 0:
                    continue
                e = flat[t0:t1].ravel() - rebase   # edge j = t*128+p order
                idx16[c, :, t0 * 8:t0 * 8 + n // 16] = (
                    e.reshape(n // 16, 16).T.astype(np.int16)
                )
    return np.tile(idx16, (1, 8, 1))  # replicate to 128 partitions


def build_sel(valsA, dstlA):
    """Per-tile selection matrices sel[e, t*128+r] = vals[e,t]*(dstl[e,t]==r),
    one [NCORES, P, Ttot*P] bf16 array streamed from DRAM on device."""
    nc_, p_, tt = valsA.shape
    r = np.arange(P, dtype=np.float32)
    out = np.empty((nc_, p_, tt * P), BF16)
    for c in range(nc_):
        onehot = (dstlA[c][:, :, None] == r)            # [P, Ttot, P]
        out[c] = (onehot * valsA[c][:, :, None]).reshape(p_, tt * P)
    return out


def _kmajor(mat, kc, width):
    """[kc*P, width] -> [P, kc*width] with layout [p, k*width + o]."""
    return np.ascontiguousarray(
        mat.reshape(kc, P, width).transpose(1, 0, 2).reshape(P, kc * width)
    )


# --------------------------------------------------------------------------
# Device program
# --------------------------------------------------------------------------

def build_program(T, TL):
    T = [int(t) for t in T]
    TL = [int(t) for t in TL]
    Ttot = sum(T)
    off = np.zeros(len(T) + 1, np.int64)
    off[1:] = np.cumsum(T)

    nc = bacc.Bacc(None, num_devices=NCORES, num_swdge_queues=NQ)
    xbf_in = nc.declare_dram_parameter("xbf", [NP, D0], BF, isOutput=False)
    sel_in = nc.declare_dram_parameter("sel", [P, Ttot * P], BF, isOutput=False)
    wc1_in = nc.declare_dram_parameter("wc1", [P, KC1 * SG1], BF, isOutput=False)
    wc2_in = nc.declare_dram_parameter("wc2", [P, KC2 * SG2], BF, isOutput=False)
    idx_in = nc.declare_dram_parameter("idx16", [P, Ttot * 8], I16, isOutput=False)
    ident_in = nc.declare_dram_parameter("ident", [P, P], F32, isOutput=False)
    out_ext = nc.declare_dram_parameter("out", [NC_ROWS, D2], F32, isOutput=True)

    rg = [list(range(NCORES))]

    MAXG = 8  # tiles per dma_gather call (HW caps one gather at 1024 idx)
    qrr = [0]  # round-robin swdge queue cursor
    dma_sems = [nc.alloc_semaphore(f"gsem{q}") for q in range(NQ)]

    def gathers(msgtile, tables, w, elem, prep=False):
        """Emit lo/hi dma_gather calls for window w into msgtile."""
        tl, th = TL[w], T[w] - TL[w]
        o = int(off[w])
        used = set()
        for (tq, tbase, src) in ((tl, 0, tables[0]),
                                 (th, tl, tables[1])):
            for c0 in range(0, tq, MAXG):
                tc_ = min(MAXG, tq - c0)
                n = tc_ * P
                b = tbase + c0
                q0 = (o + b) * 8
                q = qrr[0]
                used.add(q)
                kw = {}
                if prep:
                    kw = dict(prepare_only=True, sem=dma_sems[q])
                nc.gpsimd.dma_gather(
                    out_ap=msgtile[:, b * elem:(b + tc_) * elem].rearrange(
                        "p (t e) -> p t e", e=elem
                    ),
                    in_ap=src,
                    idxs_ap=idx_sb[:, q0:q0 + n // 16],
                    num_idxs=n,
                    num_idxs_reg=n,
                    elem_size=elem,
                    queue_num=q,
                    **kw,
                )
                qrr[0] = (qrr[0] + 1) % NQ
        return used

    with tile.TileContext(nc, num_cores=NCORES) as tc:
        with (
            tc.tile_pool(name="dram", bufs=1, space="DRAM") as dram,
            tc.tile_pool(name="const", bufs=1) as cp,
        ):
            h_own = dram.tile([NC_ROWS, D1], BF)
            h_full = dram.tile([NP, D1], BF, addr_space="Shared")

            idx_sb = cp.tile([P, Ttot * 8], I16)
            wc1_sb = cp.tile([P, KC1 * SG1], BF)
            wc2_sb = cp.tile([P, KC2 * SG2], BF)
            ident_sb = cp.tile([P, P], F32)
            nc.sync.dma_start(idx_sb[:], idx_in[:])
            nc.sync.dma_start(wc1_sb[:], wc1_in[:])
            nc.sync.dma_start(wc2_sb[:], wc2_in[:])
            nc.sync.dma_start(ident_sb[:], ident_in[:])

            def layer(tables, elem, wc_sb, sgw, dw, last, prefix=0,
                      after_window=None):
                """One GatedGCN layer; emits all 49 windows."""
                kc = elem // P
                with (
                    tc.tile_pool(name="msg", bufs=3) as msgp,
                    tc.tile_pool(name="msgpre", bufs=max(prefix, 1)) as msgpp,
                    tc.tile_pool(name="selp", bufs=3) as selp,
                    tc.tile_pool(name="selpre", bufs=max(prefix, 1)) as selpp,
                    tc.tile_pool(name="aggps", bufs=2, space="PSUM") as aggp,
                    tc.tile_pool(name="zc", bufs=2) as zcp,
                    tc.tile_pool(name="tp", bufs=2, space="PSUM") as tpp,
                    tc.tile_pool(name="zt", bufs=2) as ztp,
                    tc.tile_pool(name="dps", bufs=2, space="PSUM") as dpp,
                    tc.tile_pool(name="post", bufs=3) as postp,
                ):
                    prep_queues = set()
                    for w in range(NWIN):
                        tw = T[w]
                        o = int(off[w])
                        in_prefix = w < prefix
                        mp = msgpp if in_prefix else msgp
                        sp = selpp if in_prefix else selp
                        msg = mp.tile([P, tw * elem], BF, tag="msg")
                        prep_queues |= gathers(msg, tables, w, elem,
                                               prep=in_prefix)
                        if prefix and w == prefix - 1:
                            for q in sorted(prep_queues):
                                nc.gpsimd.trigger_dma(count=None, queue_num=q)
                        sel = sp.tile([P, tw * P], BF, tag="sel")
                        nc.sync.dma_start(
                            sel[:], sel_in[:, o * P:(o + tw) * P]
                        )
                        ps = aggp.tile([P, elem], F32, tag="aggps")
                        for t in range(tw):
                            nc.tensor.matmul(
                                ps[:],
                                lhsT=sel[:, t * P:(t + 1) * P],
                                rhs=msg[:, t * elem:(t + 1) * elem],
                                start=(t == 0),
                                stop=(t == tw - 1),
                            )
                        zc = zcp.tile([P, elem], F32, tag="zc")
                        nc.scalar.activation(
                            zc[:], ps[:], mybir.ActivationFunctionType.Copy
                        )
                        zt = ztp.tile([P, elem], BF, tag="zt")
                        for k in range(kc):
                            pt = tpp.tile([P, P], F32, tag="tp")
                            nc.tensor.transpose(
                                pt[:], zc[:, k * P:(k + 1) * P], ident_sb[:]
                            )
                            nc.vector.tensor_copy(zt[:, k * P:(k + 1) * P], pt[:])
                        psd = dpp.tile([P, sgw], F32, tag="dps")
                        for k in range(kc):
                            nc.tensor.matmul(
                                psd[:],
                                lhsT=zt[:, k * P:(k + 1) * P],
                                rhs=wc_sb[:, k * sgw:(k + 1) * sgw],
                                start=(k == 0),
                                stop=(k == kc - 1),
                            )
                        sig = postp.tile([P, dw], F32, tag="sig")
                        nc.scalar.activation(
                            sig[:], psd[:, dw:sgw],
                            mybir.ActivationFunctionType.Sigmoid,
                        )
                        if last:
                            res = postp.tile([P, dw], F32, tag="res")
                            nc.vector.tensor_mul(res[:], sig[:], psd[:, 0:dw])
                            nc.sync.dma_start(
                                out_ext[w * P:(w + 1) * P, :], res[:]
                            )
                        else:
                            prod = postp.tile([P, dw], F32, tag="prod")
                            nc.vector.tensor_mul(prod[:], sig[:], psd[:, 0:dw])
                            hbf = postp.tile([P, dw], BF, tag="hbf")
                            nc.scalar.activation(
                                hbf[:], prod[:],
                                mybir.ActivationFunctionType.Relu,
                            )
                            nc.sync.dma_start(
                                h_own[w * P:(w + 1) * P, :], hbf[:]
                            )
                        if after_window is not None:
                            after_window(w)

            layer((xbf_in[:SPLIT, :], xbf_in[SPLIT:, :]), D0, wc1_sb, SG1, D1,
                  last=False)

            nc.gpsimd.collective_compute(
                "AllGather",
                mybir.AluOpType.bypass,
                replica_groups=rg,
                ins=[h_own.opt()],
                outs=[h_full.opt()],
            )

            layer((h_full[:SPLIT, :], h_full[SPLIT:, :]), D1, wc2_sb, SG2, D2,
                  last=True, prefix=PREFIX)

    return nc


# --------------------------------------------------------------------------
# Entry point
# --------------------------------------------------------------------------

def prepare_inputs(x, rows, cols, vals, W1, G1, W2, G2):
    """Host prep: edge plan + packed per-core input maps."""
    x = np.asarray(x, np.float32)
    rows = np.asarray(rows)
    cols = np.asarray(cols)
    vals = np.asarray(vals, np.float32)
    W1 = np.asarray(W1, np.float32)
    G1 = np.asarray(G1, np.float32)
    W2 = np.asarray(W2, np.float32)
    G2 = np.asarray(G2, np.float32)

    T, TL, colsA, valsA, dstlA = build_edge_plan(rows, cols, vals)
    idx16 = build_idx16(colsA, T, TL)
    selA = build_sel(valsA, dstlA)

    x_pad = np.zeros((NP, D0), np.float32)
    x_pad[:N] = x
    xbf = x_pad.astype(BF16)
    wc1_a = _kmajor(np.concatenate([W1, G1], axis=1), KC1, SG1).astype(BF16)
    wc2_a = _kmajor(np.concatenate([W2, G2], axis=1), KC2, SG2).astype(BF16)

    in_maps = []
    for c in range(NCORES):
        in_maps.append(
            {
                "xbf": xbf,
                "sel": selA[c],
                "wc1": wc1_a,
                "wc2": wc2_a,
                "idx16": idx16[c],
                "ident": np.eye(P, dtype=np.float32),
            }
        )
    return (T, TL), in_maps


def kernel(x, rows, cols, vals, W1, G1, W2, G2):
    (T, TL), in_maps = prepare_inputs(x, rows, cols, vals, W1, G1, W2, G2)
    nc = build_program(T, TL)
    # Bacc lowering passes (register allocation, event-semaphore
    # legalization) run in finalize(); the PJRT path serializes the BIR
    # as-is, so finalize must happen before run.
    if not nc.is_finalized():
        nc.finalize()
    res = run_bass_kernel_spmd(
        nc,
        in_maps,
        list(range(NCORES)),
        trace=bool(os.environ.get("BASS_TRACE")),
    )
    global LAST_RESULTS
    LAST_RESULTS = res
    out = np.concatenate([res.results[c]["out"] for c in range(NCORES)], axis=0)
    return np.ascontiguousarray(out[:N]).astype(np.float32)


# revision 18
# speedup vs baseline: 1.2624x; 1.2624x over previous
"""GatedGCN (2-layer) Trainium2 Bass kernel, 8 NeuronCores, full-I/O contract.

Strategy (1D destination-node graph partition), v6:
  - Associativity: A@(x@W) == (A@x)@W, so each layer computes z = A@x first
    (256-wide gather+aggregation) and applies [W|G] afterwards.  Layer 1
    gathers straight from the replicated input x, so only the h AllGather
    (between the layers) remains - split in two chunks that overlap
    layer-1 compute.
  - Pad N=50000 rows to 50176 = 8 cores * 6272 rows; core c owns rows
    [c*6272, (c+1)*6272) and every edge whose DEST row lands there.
  - Gather tables are laid out chunk-major: chunk 0 holds every core's
    local windows [0,23), chunk 1 holds windows [23,49).  Each chunk is
    <32768 rows, so int16 gather indices cover it without a lo/hi split,
    and each chunk's AllGather is a plain contiguous collective into its
    own Shared tensor.  Chunk 0's AllGather fires after layer-1 window 22
    and completes while layer 1 is still running; the boundary is chosen
    so a window's chunk-0 edges fill ~8 tiles = one max-size gather call.
  - Host sorts edges by (dest window, source chunk) and pads each group
    to a uniform (across cores) number of 128-edge tiles with val=0 edges.
  - Host precomputes per-tile selection matrices
    sel[e, r] = vals[e] * (dest_local[e] == r)  (bf16, shipped as input,
    streamed from DRAM each layer) - no on-device sel builds.
  - Aggregation per window: dma_gather pulls msg = table[cols] into SBUF
    (4 SWDGE queues round-robin), then per 128-edge tile one PE matmul
    ps += sel_t.T @ msg_t accumulates z[dest, 256] in PSUM.
  - z is copied to SBUF (scalar engine), PE-transposed to zT, and the
    dense stage ps2 = zT.T @ [W|G] + sigmoid-gate product follows, all
    fused per window.
  - Layer 2 emits the chunk-0 gather calls of its first G0AHEAD windows
    before any chunk-1 call: the gpsimd queue is in-order, so this keeps
    descriptor generation running during chunk 1's AllGather instead of
    stalling behind a blocked chunk-1 gather.
"""

import os

import numpy as np
import ml_dtypes

import concourse.bass as bass
import concourse.bacc as bacc
import concourse.mybir as mybir
import concourse.tile as tile
from concourse.bass_utils import run_bass_kernel_spmd

P = 128
NCORES = 8
N = 50000
D0, D1, D2 = 256, 256, 128
NWIN = 49
NC_ROWS = NWIN * P            # 6272 rows per core (padded)
NP = NC_ROWS * NCORES         # 50176 padded rows total
SG1 = 2 * D1                  # 512: [support | gate] layer 1
SG2 = 2 * D2                  # 256: [support | gate] layer 2
KC1 = D0 // P                 # k-chunks of z for the layer-1 dense matmul
KC2 = D1 // P
NQ = 4                        # SWDGE queues, round-robin over gather calls
G0AHEAD = 10                  # layer-2 windows whose chunk-0 gathers go first

CHUNK_W = [(0, 23), (23, 49)]           # windows per gather-table chunk
CH_ROWS = [(w1 - w0) * P for (w0, w1) in CHUNK_W]   # per-core rows
CH_SIZE = [r * NCORES for r in CH_ROWS]             # table rows

BF16 = ml_dtypes.bfloat16
F32 = mybir.dt.float32
BF = mybir.dt.bfloat16
I16 = mybir.dt.int16

LAST_RESULTS = None  # test harness reads exec_time_ns from here


def chunk_pos(col):
    """Map global (padded-space) row ids -> (chunk, position in chunk table).

    Table layout per chunk g: for core c = col // NC_ROWS, local row
    r = col % NC_ROWS, rows with window(r) in CHUNK_W[g] sit at
    c * CH_ROWS[g] + (r - CHUNK_W[g][0]*P)."""
    col = np.asarray(col)
    c = col // NC_ROWS
    r = col % NC_ROWS
    g = (r >= CH_ROWS[0]).astype(np.int64)
    pos = c * np.where(g == 0, CH_ROWS[0], CH_ROWS[1]) + r - g * CH_ROWS[0]
    return g, pos


# --------------------------------------------------------------------------
# Host-side edge plan
# --------------------------------------------------------------------------

def build_edge_plan(rows, cols, vals):
    """Sort edges by (core, window) of their dest row. Within a window,
    chunk-0 edges (source in table chunk 0) come first, then chunk-1; each
    group is padded to a uniform (across cores) number of 128-edge tiles.

    Returns (T, TL, colsA, valsA, dstlA):
      T[w]   total tiles in window w;  TL[w]  chunk-0 tiles (c1 = T - TL)
      colsA  [NCORES, P, Ttot] int32   source row (global padded space)
      valsA  [NCORES, P, Ttot] float32 edge weight (0 for padding)
      dstlA  [NCORES, P, Ttot] float32 dest row local to window (0..127)
    Edge (window w, tile t, partition p) lives at [:, p, off[w]+t].
    """
    g1, _ = chunk_pos(cols)
    gw = (rows // P).astype(np.int64)               # global window
    order = np.argsort(gw * 2 + g1, kind="stable")  # window-major, chunk0 1st
    srows = rows[order]
    scols = cols[order]
    svals = vals[order]
    cnt = np.bincount(gw * 2 + g1, minlength=NCORES * NWIN * 2)
    cnt_cwh = cnt.reshape(NCORES, NWIN, 2)
    TL = np.ceil(cnt_cwh[:, :, 0] / P).astype(np.int64).max(axis=0)
    TH = np.ceil(cnt_cwh[:, :, 1] / P).astype(np.int64).max(axis=0)
    TL = np.maximum(TL, (TH == 0).astype(np.int64))  # >=1 tile per window
    T = TL + TH
    off = np.zeros(NWIN + 1, np.int64)
    off[1:] = np.cumsum(T)
    Ttot = int(off[-1])
    colsA = np.zeros((NCORES, P, Ttot), np.int32)
    # chunk-1 padding must point into the chunk-1 table: row CH_ROWS[0]
    for w in range(NWIN):
        if T[w] - TL[w] > 0:
            colsA[:, :, off[w] + TL[w]:off[w + 1]] = CH_ROWS[0]
    valsA = np.zeros((NCORES, P, Ttot), np.float32)
    dstlA = np.zeros((NCORES, P, Ttot), np.float32)
    starts = np.zeros(NCORES * NWIN * 2 + 1, np.int64)
    starts[1:] = np.cumsum(cnt)
    for c in range(NCORES):
        for w in range(NWIN):
            for h in range(2):
                g = (c * NWIN + w) * 2 + h
                s, e = int(starts[g]), int(starts[g + 1])
                n = e - s
                if n == 0:
                    continue
                base = off[w] + (0 if h == 0 else TL[w])
                j = np.arange(n)
                t_idx = j // P
                p_idx = j % P
                colsA[c, p_idx, base + t_idx] = scols[s:e]
                valsA[c, p_idx, base + t_idx] = svals[s:e]
                dstlA[c, p_idx, base + t_idx] = srows[s:e] % P
    return T, TL, colsA, valsA, dstlA


def build_idx16(colsA, T, TL):
    """Wrapped int16 gather indices (positions in the chunk tables): for
    each window w and group q (chunk0/chunk1), gather j reads
    idx16[j % 16, base*8 + j // 16]; replicated to all 128 partitions."""
    Ttot = int(np.sum(T))
    idx16 = np.zeros((NCORES, 16, Ttot * 8), np.int16)
    off = np.zeros(len(T) + 1, np.int64)
    off[1:] = np.cumsum(T)
    for c in range(NCORES):
        _, posf = chunk_pos(colsA[c].T)            # [Ttot, P] positions
        posf = np.ascontiguousarray(posf)
        for w in range(len(T)):
            for (t0, t1) in (
                (off[w], off[w] + TL[w]),
                (off[w] + TL[w], off[w + 1]),
            ):
                n = int((t1 - t0) * P)
                if n == 0:
                    continue
                e = posf[t0:t1].ravel()            # edge j = t*128+p order
                idx16[c, :, t0 * 8:t0 * 8 + n // 16] = (
                    e.reshape(n // 16, 16).T.astype(np.int16)
                )
    return np.tile(idx16, (1, 8, 1))  # replicate to 128 partitions


def build_sel(valsA, dstlA):
    """Per-tile selection matrices sel[e, t*128+r] = vals[e,t]*(dstl[e,t]==r),
    one [NCORES, P, Ttot*P] bf16 array streamed from DRAM on device."""
    nc_, p_, tt = valsA.shape
    r = np.arange(P, dtype=np.float32)
    out = np.empty((nc_, p_, tt * P), BF16)
    for c in range(nc_):
        onehot = (dstlA[c][:, :, None] == r)            # [P, Ttot, P]
        out[c] = (onehot * valsA[c][:, :, None]).reshape(p_, tt * P)
    return out


def _kmajor(mat, kc, width):
    """[kc*P, width] -> [P, kc*width] with layout [p, k*width + o]."""
    return np.ascontiguousarray(
        mat.reshape(kc, P, width).transpose(1, 0, 2).reshape(P, kc * width)
    )


# --------------------------------------------------------------------------
# Device program
# --------------------------------------------------------------------------

def build_program(T, TL):
    T = [int(t) for t in T]
    TL = [int(t) for t in TL]
    Ttot = sum(T)
    off = np.zeros(len(T) + 1, np.int64)
    off[1:] = np.cumsum(T)

    nc = bacc.Bacc(None, num_devices=NCORES, num_swdge_queues=NQ)
    xc0_in = nc.declare_dram_parameter("xc0", [CH_SIZE[0], D0], BF, isOutput=False)
    xc1_in = nc.declare_dram_parameter("xc1", [CH_SIZE[1], D0], BF, isOutput=False)
    sel_in = nc.declare_dram_parameter("sel", [P, Ttot * P], BF, isOutput=False)
    wc1_in = nc.declare_dram_parameter("wc1", [P, KC1 * SG1], BF, isOutput=False)
    wc2_in = nc.declare_dram_parameter("wc2", [P, KC2 * SG2], BF, isOutput=False)
    idx_in = nc.declare_dram_parameter("idx16", [P, Ttot * 8], I16, isOutput=False)
    ident_in = nc.declare_dram_parameter("ident", [P, P], F32, isOutput=False)
    out_ext = nc.declare_dram_parameter("out", [NC_ROWS, D2], F32, isOutput=True)

    rg = [list(range(NCORES))]

    MAXG = 8  # tiles per dma_gather call (HW caps one gather at 1024 idx)
    qrr = [0]  # round-robin swdge queue cursor

    def gather_group(msgtile, src, w, elem, grp):
        """Emit the gather calls of one (window, chunk-group)."""
        tl, th = TL[w], T[w] - TL[w]
        (tq, tbase) = (tl, 0) if grp == 0 else (th, tl)
        o = int(off[w])
        for c0 in range(0, tq, MAXG):
            tc_ = min(MAXG, tq - c0)
            n = tc_ * P
            b = tbase + c0
            q0 = (o + b) * 8
            nc.gpsimd.dma_gather(
                out_ap=msgtile[:, b * elem:(b + tc_) * elem].rearrange(
                    "p (t e) -> p t e", e=elem
                ),
                in_ap=src,
                idxs_ap=idx_sb[:, q0:q0 + n // 16],
                num_idxs=n,
                num_idxs_reg=n,
                elem_size=elem,
                queue_num=qrr[0],
            )
            qrr[0] = (qrr[0] + 1) % NQ

    with tile.TileContext(nc, num_cores=NCORES) as tc:
        with (
            tc.tile_pool(name="dram", bufs=1, space="DRAM") as dram,
            tc.tile_pool(name="const", bufs=1) as cp,
        ):
            h_own0 = dram.tile([CH_ROWS[0], D1], BF)
            h_own1 = dram.tile([CH_ROWS[1], D1], BF)
            h_full0 = dram.tile([CH_SIZE[0], D1], BF, addr_space="Shared")
            h_full1 = dram.tile([CH_SIZE[1], D1], BF, addr_space="Shared")
            h_own = [h_own0, h_own1]
            h_full = [h_full0, h_full1]

            idx_sb = cp.tile([P, Ttot * 8], I16)
            wc1_sb = cp.tile([P, KC1 * SG1], BF)
            wc2_sb = cp.tile([P, KC2 * SG2], BF)
            ident_sb = cp.tile([P, P], F32)
            nc.sync.dma_start(idx_sb[:], idx_in[:])
            nc.sync.dma_start(wc1_sb[:], wc1_in[:])
            nc.sync.dma_start(wc2_sb[:], wc2_in[:])
            nc.sync.dma_start(ident_sb[:], ident_in[:])

            def layer(tables, elem, wc_sb, sgw, dw, last, g0ahead=0,
                      after_window=None):
                """One GatedGCN layer; emits all 49 windows."""
                kc = elem // P
                with (
                    tc.tile_pool(name="msg", bufs=3) as msgp,
                    tc.tile_pool(name="msgpre", bufs=max(g0ahead, 1)) as msgpp,
                    tc.tile_pool(name="selp", bufs=3) as selp,
                    tc.tile_pool(name="aggps", bufs=2, space="PSUM") as aggp,
                    tc.tile_pool(name="zc", bufs=2) as zcp,
                    tc.tile_pool(name="tp", bufs=2, space="PSUM") as tpp,
                    tc.tile_pool(name="zt", bufs=2) as ztp,
                    tc.tile_pool(name="dps", bufs=2, space="PSUM") as dpp,
                    tc.tile_pool(name="post", bufs=3) as postp,
                ):
                    # pre-create the lookahead windows' msg tiles and emit
                    # their chunk-0 gathers before any chunk-1 call: keeps
                    # gpsimd desc-gen busy while chunk-1's AllGather runs.
                    msgs = {}
                    for w in range(min(g0ahead, NWIN)):
                        m = msgpp.tile([P, T[w] * elem], BF, tag="msgpre")
                        msgs[w] = m
                        gather_group(m, tables[0], w, elem, 0)
                    for w in range(NWIN):
                        tw = T[w]
                        o = int(off[w])
                        if w in msgs:
                            msg = msgs[w]
                            gather_group(msg, tables[1], w, elem, 1)
                        else:
                            msg = msgp.tile([P, tw * elem], BF, tag="msg")
                            gather_group(msg, tables[0], w, elem, 0)
                            gather_group(msg, tables[1], w, elem, 1)
                        sel = selp.tile([P, tw * P], BF, tag="sel")
                        nc.sync.dma_start(
                            sel[:], sel_in[:, o * P:(o + tw) * P]
                        )
                        ps = aggp.tile([P, elem], F32, tag="aggps")
                        for t in range(tw):
                            nc.tensor.matmul(
                                ps[:],
                                lhsT=sel[:, t * P:(t + 1) * P],
                                rhs=msg[:, t * elem:(t + 1) * elem],
                                start=(t == 0),
                                stop=(t == tw - 1),
                            )
                        zc = zcp.tile([P, elem], F32, tag="zc")
                        nc.scalar.activation(
                            zc[:], ps[:], mybir.ActivationFunctionType.Copy
                        )
                        zt = ztp.tile([P, elem], BF, tag="zt")
                        for k in range(kc):
                            pt = tpp.tile([P, P], F32, tag="tp")
                            nc.tensor.transpose(
                                pt[:], zc[:, k * P:(k + 1) * P], ident_sb[:]
                            )
                            nc.vector.tensor_copy(zt[:, k * P:(k + 1) * P], pt[:])
                        psd = dpp.tile([P, sgw], F32, tag="dps")
                        for k in range(kc):
                            nc.tensor.matmul(
                                psd[:],
                                lhsT=zt[:, k * P:(k + 1) * P],
                                rhs=wc_sb[:, k * sgw:(k + 1) * sgw],
                                start=(k == 0),
                                stop=(k == kc - 1),
                            )
                        sig = postp.tile([P, dw], F32, tag="sig")
                        nc.scalar.activation(
                            sig[:], psd[:, dw:sgw],
                            mybir.ActivationFunctionType.Sigmoid,
                        )
                        if last:
                            res = postp.tile([P, dw], F32, tag="res")
                            nc.vector.tensor_mul(res[:], sig[:], psd[:, 0:dw])
                            nc.sync.dma_start(
                                out_ext[w * P:(w + 1) * P, :], res[:]
                            )
                        else:
                            prod = postp.tile([P, dw], F32, tag="prod")
                            nc.vector.tensor_mul(prod[:], sig[:], psd[:, 0:dw])
                            hbf = postp.tile([P, dw], BF, tag="hbf")
                            nc.scalar.activation(
                                hbf[:], prod[:],
                                mybir.ActivationFunctionType.Relu,
                            )
                            g = 0 if w < CHUNK_W[0][1] else 1
                            r0 = (w - CHUNK_W[g][0]) * P
                            nc.sync.dma_start(
                                h_own[g][r0:r0 + P, :], hbf[:]
                            )
                        if after_window is not None:
                            after_window(w)

            def l1_after(w):
                for g, (w0, w1) in enumerate(CHUNK_W):
                    if w == w1 - 1:
                        nc.gpsimd.collective_compute(
                            "AllGather",
                            mybir.AluOpType.bypass,
                            replica_groups=rg,
                            ins=[h_own[g].opt()],
                            outs=[h_full[g].opt()],
                        )

            layer((xc0_in[:], xc1_in[:]), D0, wc1_sb, SG1, D1, last=False,
                  after_window=l1_after)

            layer((h_full0[:], h_full1[:]), D1, wc2_sb, SG2, D2,
                  last=True, g0ahead=G0AHEAD)

    return nc


# --------------------------------------------------------------------------
# Entry point
# --------------------------------------------------------------------------

def prepare_inputs(x, rows, cols, vals, W1, G1, W2, G2):
    """Host prep: edge plan + packed per-core input maps."""
    x = np.asarray(x, np.float32)
    rows = np.asarray(rows)
    cols = np.asarray(cols)
    vals = np.asarray(vals, np.float32)
    W1 = np.asarray(W1, np.float32)
    G1 = np.asarray(G1, np.float32)
    W2 = np.asarray(W2, np.float32)
    G2 = np.asarray(G2, np.float32)

    T, TL, colsA, valsA, dstlA = build_edge_plan(rows, cols, vals)
    idx16 = build_idx16(colsA, T, TL)
    selA = build_sel(valsA, dstlA)

    x_pad = np.zeros((NP, D0), np.float32)
    x_pad[:N] = x
    xbf = x_pad.astype(BF16)
    # repack into chunk-major table layout (see chunk_pos)
    xr = xbf.reshape(NCORES, NC_ROWS, D0)
    xc0 = np.ascontiguousarray(xr[:, :CH_ROWS[0], :]).reshape(CH_SIZE[0], D0)
    xc1 = np.ascontiguousarray(xr[:, CH_ROWS[0]:, :]).reshape(CH_SIZE[1], D0)
    wc1_a = _kmajor(np.concatenate([W1, G1], axis=1), KC1, SG1).astype(BF16)
    wc2_a = _kmajor(np.concatenate([W2, G2], axis=1), KC2, SG2).astype(BF16)

    in_maps = []
    for c in range(NCORES):
        in_maps.append(
            {
                "xc0": xc0,
                "xc1": xc1,
                "sel": selA[c],
                "wc1": wc1_a,
                "wc2": wc2_a,
                "idx16": idx16[c],
                "ident": np.eye(P, dtype=np.float32),
            }
        )
    return (T, TL), in_maps


def kernel(x, rows, cols, vals, W1, G1, W2, G2):
    (T, TL), in_maps = prepare_inputs(x, rows, cols, vals, W1, G1, W2, G2)
    nc = build_program(T, TL)
    # Bacc lowering passes (register allocation, event-semaphore
    # legalization) run in finalize(); the PJRT path serializes the BIR
    # as-is, so finalize must happen before run.
    if not nc.is_finalized():
        nc.finalize()
    res = run_bass_kernel_spmd(
        nc,
        in_maps,
        list(range(NCORES)),
        trace=bool(os.environ.get("BASS_TRACE")),
    )
    global LAST_RESULTS
    LAST_RESULTS = res
    out = np.concatenate([res.results[c]["out"] for c in range(NCORES)], axis=0)
    return np.ascontiguousarray(out[:N]).astype(np.float32)
